# revision 19
# baseline (speedup 1.0000x reference)
"""GPT2-style decision-transformer forward pass on 8 TRN2 NeuronCores.

Data-parallel: 16 sequences -> 2 per core. Each core runs the full
4-layer transformer on its 2 sequences (602 tokens) and reduces its
loss-sum / correct-count to a [1,2] output; the host sums the 8 partials.

v2: all large matmuls (qkv, v, fc, mproj, aproj) run in fp8e4 with
DoubleRow perf mode (two 128-deep k-tiles per matmul, 2x PE throughput).
Weights are host-scaled by WS=16 (keeps small entries out of the fp8
subnormal range) and the 1/WS is folded into the psum->sbuf copy.
Activations feeding fp8 matmuls (xnT, attnT, geluT) are stored as fp8
"pair" tiles [128, 2, T] where dim1 indexes the two k-tiles of a
DoubleRow pair.  Attention (scores, softmax, PV) stays bf16.

All problem biases are zero (setup_inputs uses zeros); the bias-via-
ones-matmul paths are only emitted when the host detects a nonzero
bias (build is cached per flag set).  qkv/fc biases ride for free in
the psum->sbuf copy ops either way.

Engine split per layer: PE matmuls; ACT xnorm/sqrt/exp/gelu + some
copies; DVE stats/copies/residual adds; GpSimd builds the softmax
1/rowsum diag matrices (SBUF-only engine, otherwise idle).
"""

import numpy as np
import ml_dtypes
from contextlib import ExitStack

import concourse.bass as bass
import concourse.tile as tile
from concourse import bacc, mybir
from concourse.bass_utils import run_bass_kernel_spmd

F32 = mybir.dt.float32
BF16 = mybir.dt.bfloat16
FP8 = mybir.dt.float8e4
AF = mybir.ActivationFunctionType
ALU = mybir.AluOpType
DR = mybir.MatmulPerfMode.DoubleRow

B, CTX, D, H, NL, DFF, G, NA = 16, 100, 1024, 16, 4, 4096, 9, 5
L = 3 * CTX + 1          # 301
HD = D // H              # 64
LN_EPS = 1e-5
LS = 0.1
NCORES = 8
S = B // NCORES          # 2 seqs per core
LP = 304                 # per-seq padded length (4-aligned for fp8 lhsT)
T = S * LP               # 608 padded tokens per core
NTOK = [128, 128, 128, 128, 96]   # token tile sizes (padded axis)
NT = len(NTOK)
EMB = G * G + NA + 1     # 87 combined embedding rows
NEG = -60.0              # additive causal mask value (exp(-60) ~ 9e-27)
WS = 16.0                # fp8 weight pre-scale
IWS = 1.0 / WS


def _bf16(x):
    return np.asarray(x, dtype=ml_dtypes.bfloat16)


def _f8(x):
    return np.asarray(x, dtype=ml_dtypes.float8_e4m3)


def _f32(x):
    return np.ascontiguousarray(np.asarray(x, dtype=np.float32))


def _pack_pairs(w, nkp, nfg):
    """[K, F] -> [nkp, 128, nfg, 2, 512] DoubleRow pair layout.
    pack[kp, p, fg, j, f] = w[(2*kp + j)*128 + p, fg*512 + f]."""
    K, F = w.shape
    assert K == nkp * 256 and F == nfg * 512
    return np.ascontiguousarray(
        w.reshape(nkp, 2, 128, nfg, 512).transpose(0, 2, 3, 1, 4))


# --------------------------------------------------------------------------
# graph builder
# --------------------------------------------------------------------------

def build(flags=(False, False, False, False)):
    vbias_f, apbias_f, mpbias_f, predbias_f = flags
    any_ones = vbias_f or apbias_f or mpbias_f or predbias_f
    nc = bacc.Bacc("TRN2", target_bir_lowering=False, debug=False,
                   enable_asserts=True, num_devices=NCORES)

    def inp(name, shape, dt):
        return nc.dram_tensor(name, list(shape), dt, kind="ExternalInput").ap()

    d_membT = inp("m_embT", (EMB, T), BF16)
    d_temb = inp("t_emb", (EMB, D), BF16)
    d_wpe = inp("wpe", (T, D), F32)
    d_wqkv, d_bqkv, d_wap = [], [], []
    d_wfc, d_bfc, d_wmp = [], [], []
    d_bv, d_bap, d_bmp = [], [], []
    for i in range(NL):
        d_wqkv.append(inp(f"w_qkv_{i}", (4, 128, 6, 2, 512), FP8))
        d_bqkv.append(inp(f"b_qkv_{i}", (128, 16), F32))
        d_wap.append(inp(f"w_aproj_{i}", (4, 128, 2, 2, 512), FP8))
        d_wfc.append(inp(f"w_fc_{i}", (4, 128, 8, 2, 512), FP8))
        d_bfc.append(inp(f"b_fc_{i}", (128, 32), F32))
        d_wmp.append(inp(f"w_mproj_{i}", (16, 128, 2, 2, 512), FP8))
        if vbias_f:
            d_bv.append(inp(f"b_v_{i}", (1, D), BF16))
        if apbias_f:
            d_bap.append(inp(f"b_aproj_{i}", (1, D), BF16))
        if mpbias_f:
            d_bmp.append(inp(f"b_mproj_{i}", (1, D), BF16))
    d_wpred = inp("w_pred", (128, 8, NA), BF16)   # host pre-laid-out
    if predbias_f:
        d_bpred = inp("b_pred", (1, NA), BF16)
    d_tgt = inp("tgt_oh", (T, NA), F32)
    d_smask = inp("smask", (128, NT), F32)
    d_ident8 = inp("ident_f8", (128, 128), FP8)
    d_ident5 = inp("ident5", (NA, NA), F32)
    d_tri = inp("tri", (128, 128), F32)
    if any_ones:
        d_ones1 = inp("ones1", (1, 512), BF16)
    d_ones128 = inp("ones128", (128, 1), F32)
    d_out = nc.dram_tensor("out", [1, 2], F32, kind="ExternalOutput").ap()

    with tile.TileContext(nc) as tc, ExitStack() as ctx:
        # ---------------- pools
        const_p = ctx.enter_context(tc.tile_pool(name="const", bufs=1))
        pers_p = ctx.enter_context(tc.tile_pool(name="pers", bufs=1))
        w_p = ctx.enter_context(tc.tile_pool(name="w", bufs=36))
        bias_p = ctx.enter_context(tc.tile_pool(name="bias", bufs=2))
        xn_p = ctx.enter_context(tc.tile_pool(name="xn", bufs=3))
        st_p = ctx.enter_context(tc.tile_pool(name="st", bufs=24))
        pr_p = ctx.enter_context(tc.tile_pool(name="pr", bufs=28))
        ps_p = ctx.enter_context(tc.tile_pool(name="ps", bufs=8, space="PSUM"))

        def psum(pdim=128, fdim=512, dt=F32):
            t = ps_p.tile([128, 512], F32, tag="ps")
            return t[:pdim, :fdim]

        # ---------------- constants
        ident8 = const_p.tile([128, 128], FP8, tag="ident8")
        nc.sync.dma_start(ident8[:], d_ident8[:, :])
        ident5 = const_p.tile([NA, NA], F32, tag="ident5")
        nc.sync.dma_start(ident5[:], d_ident5[:, :])
        tri = const_p.tile([128, 128], F32, tag="tri")
        nc.sync.dma_start(tri[:], d_tri[:, :])
        if any_ones:
            ones1 = const_p.tile([1, 512], BF16, tag="ones1")
            nc.sync.dma_start(ones1[:], d_ones1[:, :])
        ones128 = const_p.tile([128, 1], F32, tag="ones128")
        nc.sync.dma_start(ones128[:], d_ones128[:, :])
        if predbias_f:
            bpred = const_p.tile([1, NA], BF16, tag="bpred")
            nc.sync.dma_start(bpred[:], d_bpred[:, :])
        smask = const_p.tile([128, NT], F32, tag="smask")
        nc.sync.dma_start(smask[:], d_smask[:, :])
        wpred = const_p.tile([128, 8, NA], BF16, tag="wpred")
        nc.sync.dma_start(wpred[:], d_wpred[:, :, :])
        tgt = const_p.tile([128, NT, NA], F32, tag="tgt")
        for tt in range(NT):
            n = NTOK[tt]
            nc.sync.dma_start(tgt[:n, tt, :], d_tgt[tt * 128:tt * 128 + n, :])
        eps_sb = const_p.tile([128, 1], F32, tag="eps")
        nc.vector.memset(eps_sb[:], LN_EPS)
        membT = const_p.tile([EMB, T], BF16, tag="membT")
        nc.sync.dma_start(membT[:], d_membT[:, :])
        temb = const_p.tile([EMB, D], BF16, tag="temb")
        nc.sync.dma_start(temb[:], d_temb[:, :])

        # ---------------- persistent activations
        h = [pers_p.tile([128, D], F32, tag=f"h{i}", name=f"h{i}")
             for i in range(NT)]
        # fp8 pair tiles: dim1 indexes the two k-tiles of a DoubleRow pair
        xnT = [pers_p.tile([128, 2, T], FP8, tag=f"xnT{i}", name=f"xnT{i}")
               for i in range(4)]
        qkvT = [pers_p.tile([128, T], BF16, tag=f"qkvT{i}", name=f"qkvT{i}")
                for i in range(16)]
        # v with a ones column per head (col 64): PV row-sums ride for free
        vsb = [pers_p.tile([128, 16, 65], BF16, tag=f"vsb{i}", name=f"vsb{i}")
               for i in range(6)]
        for i in range(6):
            nc.vector.memset(vsb[i][:, :, 64:65], 1.0)
        attnT = [pers_p.tile([128, 2, T], FP8, tag=f"attnT{i}", name=f"attnT{i}")
                 for i in range(4)]
        geluT = [pers_p.tile([128, 2, T], FP8, tag=f"geluT{i}", name=f"geluT{i}")
                 for i in range(16)]

        # attnT pad columns are never written by attention; zero them once
        for pp in range(4):
            nc.vector.memset(attnT[pp][:, :, L:LP], 0.0)
            nc.vector.memset(attnT[pp][:, :, LP + L:], 0.0)

        # ---------------- embedding: h = wpe_eff + M_embT.T @ T_emb
        for tt in range(NT):
            n = NTOK[tt]
            nc.sync.dma_start(h[tt][:n, :], d_wpe[tt * 128:tt * 128 + n, :])
            for half in range(2):
                ps = psum(n, 512)
                nc.tensor.matmul(ps, membT[:, tt * 128:tt * 128 + n],
                                 temb[:, half * 512:(half + 1) * 512],
                                 start=True, stop=True)
                nc.vector.tensor_tensor(
                    out=h[tt][:n, half * 512:(half + 1) * 512],
                    in0=h[tt][:n, half * 512:(half + 1) * 512],
                    in1=ps, op=ALU.add)

        # ---------------- helpers
        def ln_tile(tt, to_qkvT=False):
            """LN (pure normalize) on h[tt] -> xn fp8 -> transpose into the
            xnT fp8 pair tiles (or bf16 qkvT tiles for the final LN)."""
            n = NTOK[tt]
            st6 = st_p.tile([128, 2, 6], F32, tag="st6")
            mv = st_p.tile([128, 2], F32, tag="mv")
            std = st_p.tile([128, 1], F32, tag="std")
            inv = st_p.tile([128, 1], F32, tag="inv")
            nmi = st_p.tile([128, 1], F32, tag="nmi")
            nc.vector.bn_stats(out=st6[:n, 0, :], in_=h[tt][:n, 0:512])
            nc.vector.bn_stats(out=st6[:n, 1, :], in_=h[tt][:n, 512:1024])
            nc.vector.bn_aggr(out=mv[:n, :], in_=st6[:n, :, :])
            nc.scalar.activation(out=std[:n, :], in_=mv[:n, 1:2],
                                 func=AF.Sqrt, bias=eps_sb[:n, :], scale=1.0)
            nc.vector.reciprocal(inv[:n, :], std[:n, :])
            nc.vector.scalar_tensor_tensor(
                out=nmi[:n, :], in0=mv[:n, 0:1], scalar=-1.0,
                in1=inv[:n, :], op0=ALU.mult, op1=ALU.mult)
            xn = xn_p.tile([128, D], FP8, tag="xn")
            nc.scalar.activation(out=xn[:n, :], in_=h[tt][:n, :],
                                 func=AF.Identity, bias=nmi[:n, :],
                                 scale=inv[:n, :])
            for pp in range(4):
                # both halves of a pair transposed into one psum bank
                ps3 = ps_p.tile([128, 2, 256], F32, tag="ps", name="ps_tr")
                for j in range(2):
                    dc = 2 * pp + j
                    nc.tensor.matmul(ps3[:, j, :n],
                                     xn[:n, dc * 128:(dc + 1) * 128],
                                     ident8[:n, :n],
                                     start=(j == 0), stop=(j == 1),
                                     skip_group_check=True)
                c0 = tt * 128
                if to_qkvT:
                    for j in range(2):
                        nc.vector.tensor_copy(
                            out=qkvT[2 * pp + j][:, c0:c0 + n],
                            in_=ps3[:, j, :n])
                elif pp % 2 == 0:
                    nc.vector.tensor_copy(out=xnT[pp][:, :, c0:c0 + n],
                                          in_=ps3[:, :, :n])
                else:
                    nc.scalar.copy(out=xnT[pp][:, :, c0:c0 + n],
                                   in_=ps3[:, :, :n])

        def featT_group_weights(d_w, fg):
            wt = []
            for kp in range(4):
                w = w_p.tile([128, 2, 512], FP8, tag="w")
                nc.sync.dma_start(w[:], d_w[kp, :, fg, :, :])
                wt.append(w)
            return wt

        def featT_group_mms(wt, outT, fg, bias_sb, act_func, fs_range):
            """DoubleRow featT matmuls: out f-tile fq = 4*fg+fs."""
            for fs in fs_range:
                fq = fg * 4 + fs
                ps0 = psum(128, 512)
                ps1 = psum(128, 96)
                for kp in range(4):
                    lhs = wt[kp][:, :, fs * 128:(fs + 1) * 128]
                    nc.tensor.matmul(ps0, lhs, xnT[kp][:, :, 0:512],
                                     start=(kp == 0), stop=(kp == 3),
                                     perf_mode=DR)
                    nc.tensor.matmul(ps1, lhs, xnT[kp][:, :, 512:608],
                                     start=(kp == 0), stop=(kp == 3),
                                     perf_mode=DR)
                for ps, sl in ((ps0, slice(0, 512)), (ps1, slice(512, 608))):
                    if act_func is None:
                        nc.vector.tensor_scalar(
                            out=outT[fq][:, sl], in0=ps,
                            scalar1=IWS, scalar2=bias_sb[:, fq:fq + 1],
                            op0=ALU.mult, op1=ALU.add)
                    else:
                        # fc path: out tile fq of geluT pairs
                        nc.scalar.activation(
                            out=outT[fq // 2][:, fq % 2, sl], in_=ps,
                            func=act_func, bias=bias_sb[:, fq:fq + 1],
                            scale=IWS)

        def proj_residual(d_w, inT, nkp, tail=None, bias_sb=None):
            """h += (inT.T @ W) / WS [+ b];  inT fp8 pair tiles."""
            for nh in range(2):
                pss = [psum(NTOK[tt], 512) for tt in range(NT)]
                for blk in range(0, nkp, 8):
                    be = min(blk + 8, nkp)
                    wt = []
                    for kp in range(blk, be):
                        w = w_p.tile([128, 2, 512], FP8, tag="w")
                        nc.sync.dma_start(w[:], d_w[kp, :, nh, :, :])
                        wt.append(w)
                    for tt in range(NT):
                        n = NTOK[tt]
                        for j, kp in enumerate(range(blk, be)):
                            nc.tensor.matmul(
                                pss[tt],
                                inT[kp][:, :, tt * 128:tt * 128 + n],
                                wt[j], start=(kp == 0), stop=(kp == nkp - 1),
                                perf_mode=DR)
                for tt in range(NT):
                    n = NTOK[tt]
                    if bias_sb is not None:
                        # rarely-taken generic path: bias via ones matmul
                        nc.tensor.matmul(pss[tt], ones1[0:1, :n],
                                         bias_sb[0:1, nh * 512:(nh + 1) * 512],
                                         start=False, stop=True,
                                         skip_group_check=True)
                    nc.vector.scalar_tensor_tensor(
                        out=h[tt][:n, nh * 512:(nh + 1) * 512],
                        in0=pss[tt], scalar=IWS,
                        in1=h[tt][:n, nh * 512:(nh + 1) * 512],
                        op0=ALU.mult, op1=ALU.add)
                    if nh == 1 and tail is not None:
                        tail(tt)

        # ---------------- transformer layers
        SEQCH = [(0, 128), (128, 128), (256, 45)]   # per-seq k-chunks

        for li in range(NL):
            bqkv = bias_p.tile([128, 16], F32, tag="bqkv")
            nc.sync.dma_start(bqkv[:], d_bqkv[li][:, :])
            bfc = bias_p.tile([128, 32], F32, tag="bfc")
            nc.sync.dma_start(bfc[:], d_bfc[li][:, :])
            bv = bap = bmp = None
            if vbias_f:
                bv = bias_p.tile([1, D], BF16, tag="bv")
                nc.sync.dma_start(bv[:], d_bv[li][:, :])
            if apbias_f:
                bap = bias_p.tile([1, D], BF16, tag="bap")
                nc.sync.dma_start(bap[:], d_bap[li][:, :])
            if mpbias_f:
                bmp = bias_p.tile([1, D], BF16, tag="bmp")
                nc.sync.dma_start(bmp[:], d_bmp[li][:, :])

            # ---- ln1; v first (swapped DoubleRow matmul producing
            # V[tok, feat] per-seq-chunk), then q,k via featT matmuls
            if li == 0:
                for tt in range(NT):
                    ln_tile(tt)
            for nh in range(2):
                wv = []
                for kp in range(4):
                    w = w_p.tile([128, 2, 512], FP8, tag="w", name="wv")
                    nc.sync.dma_start(w[:], d_wqkv[li][kp, :, 4 + nh, :, :])
                    wv.append(w)
                for s in range(2):
                    for kc in range(3):
                        c0, cw = SEQCH[kc]
                        col = s * LP + c0
                        ps = psum(cw, 512)
                        for kp in range(4):
                            nc.tensor.matmul(ps, xnT[kp][:, :, col:col + cw],
                                             wv[kp], start=(kp == 0),
                                             stop=(kp == 3), perf_mode=DR)
                        if vbias_f:
                            nc.tensor.matmul(ps, ones1[0:1, :cw],
                                             bv[0:1, nh * 512:(nh + 1) * 512],
                                             start=False, stop=True,
                                             skip_group_check=True)
                        nc.vector.tensor_scalar_mul(
                            out=vsb[s * 3 + kc][:cw, nh * 8:(nh + 1) * 8, 0:64],
                            in0=ps.rearrange("p (h f) -> p h f", h=8),
                            scalar1=IWS)

            # ---- attention (transposed-scores form), woven with the qkv
            # matmul groups: head-pair hp needs only qkvT tiles 2hp, 2hp+1.
            # scoresT[k, q] = K^T Q per k-chunk; exp -> probsT (bf16, no
            # accumulator); PV-flipped out[q, 65] = probsT^T @ [V | 1] gives
            # the row sums for free in column 64; per-partition normalize on
            # the psum->sbuf copy; fp8 transpose-back into attnT pairs.
            QWS = [128, 128, 45]

            def attn_scores(hp, s):
                """scoresT + exp for both heads of pair hp, seq s."""
                off = s * LP
                probsT = {}
                for kc in range(3):
                    kcw = QWS[kc]
                    qn = L - kc * 128      # q range [kc*128, L)
                    for sub in range(2):
                        bp = sub * 64
                        ps_s = psum(kcw, qn)
                        nc.tensor.matmul(
                            ps_s,
                            qkvT[2 * hp + 1][bp:bp + 64,
                                             off + kc * 128: off + kc * 128 + kcw],
                            qkvT[2 * hp][bp:bp + 64, off + kc * 128: off + L],
                            start=True, stop=True)
                        nc.vector.tensor_tensor(
                            out=ps_s[:, 0:kcw], in0=ps_s[:, 0:kcw],
                            in1=tri[:kcw, :kcw], op=ALU.add)
                        pb = pr_p.tile([128, L], BF16, tag="probs")
                        nc.scalar.activation(out=pb[:kcw, :qn], in_=ps_s,
                                             func=AF.Exp)
                        probsT[sub, kc] = pb
                return (probsT,)

            def attn_pt_pv(hp, s, probsT):
                """flipped PV + rowsum, normalize, transpose-back; hp, seq s."""
                off = s * LP
                # PV: one bank per sub holding 3 q-chunks of [qw, 65]
                avs, rsi, aq = [], [], {}
                for sub in range(2):
                    av = ps_p.tile([128, 3, 65], F32, tag="ps", name="ps_av")
                    for qc in range(3):
                        qw = QWS[qc]
                        for kc in range(qc + 1):
                            kcw = QWS[kc]
                            lo = (qc - kc) * 128
                            nc.tensor.matmul(
                                av[:qw, qc, :],
                                probsT[sub, kc][:kcw, lo:lo + qw],
                                vsb[s * 3 + kc][:kcw, 2 * hp + sub, :],
                                start=(qc == 0 and kc == 0), stop=(kc == qc),
                                skip_group_check=True)
                    rs = st_p.tile([128, 3], F32, tag="rs")
                    nc.vector.tensor_copy(out=rs[:, :], in_=av[:, :, 64])
                    ri = st_p.tile([128, 3], F32, tag="ri")
                    nc.vector.reciprocal(ri[:, :], rs[:, :])
                    avs.append(av)
                    rsi.append(ri)
                    for qc in range(3):
                        qw = QWS[qc]
                        a = pr_p.tile([128, 64], FP8, tag="aq")
                        nc.scalar.activation(out=a[:qw, :],
                                             in_=av[:qw, qc, 0:64],
                                             func=AF.Copy,
                                             scale=rsi[sub][:qw, qc:qc + 1])
                        aq[sub, qc] = a
                # transpose-back: one bank, sub0 at partitions 0:64, sub1 at
                # 64:128; q-chunks at disjoint columns (start only on first)
                tb = ps_p.tile([128, 384], F32, tag="ps", name="ps_tb")
                for sub in range(2):
                    bp = sub * 64
                    for qc in range(3):
                        qw = QWS[qc]
                        nc.tensor.matmul(
                            tb[bp:bp + 64, qc * 128: qc * 128 + qw],
                            aq[sub, qc][:qw, :], ident8[:qw, :qw],
                            start=(qc == 0), stop=(qc == 2),
                            skip_group_check=True)
                # tb columns 0:301 are exactly q 0:301 (128-wide slots align)
                nc.vector.tensor_copy(
                    out=attnT[hp // 2][:, hp % 2, off:off + L],
                    in_=tb[:, 0:L])

            wt = featT_group_weights(d_wqkv[li], 0)
            featT_group_mms(wt, qkvT, 0, bqkv, None, range(4))
            pend = None   # software-pipeline pt/pv one head-pair behind
            for fg in range(4):
                for j, hp in enumerate((2 * fg, 2 * fg + 1)):
                    st0 = attn_scores(hp, 0)
                    st1 = attn_scores(hp, 1)
                    # interleave half of next qkv group's matmuls
                    if fg < 3:
                        if j == 0:
                            wt = featT_group_weights(d_wqkv[li], fg + 1)
                            featT_group_mms(wt, qkvT, fg + 1, bqkv, None, (0, 1))
                        else:
                            featT_group_mms(wt, qkvT, fg + 1, bqkv, None, (2, 3))
                    if pend is not None:
                        attn_pt_pv(*pend[0])
                        attn_pt_pv(*pend[1])
                    pend = ((hp, 0) + st0, (hp, 1) + st1)
            attn_pt_pv(*pend[0])
            attn_pt_pv(*pend[1])

            # ---- attn proj + residual (ln2 fused into the tail)
            proj_residual(d_wap[li], attnT, 4, tail=ln_tile,
                          bias_sb=bap if apbias_f else None)

            # ---- mlp (next-layer ln1 / lnf fused into mproj's tail)
            for fg in range(8):
                wt = featT_group_weights(d_wfc[li], fg)
                featT_group_mms(wt, geluT, fg, bfc, AF.Gelu_apprx_tanh,
                                range(4))
            last = (li == NL - 1)
            proj_residual(d_wmp[li], geluT, 16,
                          tail=(lambda tt: ln_tile(tt, to_qkvT=True))
                          if last else ln_tile,
                          bias_sb=bmp if mpbias_f else None)

        # ---------------- logits + loss (lnf output sits in qkvT bf16)
        lg = const_p.tile([NA, T], F32, tag="logits")
        for tch, (t0, tn) in enumerate(((0, 512), (512, 96))):
            ps = psum(NA, tn)
            for dc in range(8):
                nc.tensor.matmul(ps, wpred[:, dc, :], qkvT[dc][:, t0:t0 + tn],
                                 start=(dc == 0), stop=(dc == 7))
            if predbias_f:
                nc.tensor.matmul(ps, bpred[0:1, :], ones1[0:1, :tn],
                                 start=False, stop=True,
                                 skip_group_check=True)
            nc.scalar.copy(out=lg[:, t0:t0 + tn], in_=ps)

        ps_out = psum(1, 2)
        for tt in range(NT):
            n = NTOK[tt]
            ps_t = psum(n, NA)
            nc.tensor.matmul(ps_t, lg[:, tt * 128:tt * 128 + n],
                             ident5[:, :], start=True, stop=True)
            sexp = st_p.tile([128, 1], F32, tag="sexp")
            lse = st_p.tile([128, 1], F32, tag="lse")
            lt = st_p.tile([128, 1], F32, tag="lt")
            suml = st_p.tile([128, 1], F32, tag="suml")
            mx = st_p.tile([128, 1], F32, tag="mx")
            t1 = st_p.tile([128, 1], F32, tag="t1")
            lossv = st_p.tile([128, 1], F32, tag="lossv")
            corr = st_p.tile([128, 1], F32, tag="corr")
            scr5 = st_p.tile([128, NA], F32, tag="scr5")
            scr5b = st_p.tile([128, NA], F32, tag="scr5b")
            nc.scalar.activation(out=scr5[:n, :], in_=ps_t, func=AF.Exp,
                                 accum_out=sexp[:n, :])
            nc.scalar.activation(out=lse[:n, :], in_=sexp[:n, :], func=AF.Ln)
            nc.vector.scalar_tensor_tensor(
                out=scr5b[:n, :], in0=ps_t, scalar=1.0,
                in1=tgt[:n, tt, :], op0=ALU.mult, op1=ALU.mult,
                accum_out=lt[:n, :])
            nc.vector.tensor_reduce(out=suml[:n, :], in_=ps_t,
                                    axis=mybir.AxisListType.X, op=ALU.add)
            nc.vector.tensor_reduce(out=mx[:n, :], in_=ps_t,
                                    axis=mybir.AxisListType.X, op=ALU.max)
            nc.vector.scalar_tensor_tensor(
                out=t1[:n, :], in0=lt[:n, :], scalar=-(1.0 - LS),
                in1=lse[:n, :], op0=ALU.mult, op1=ALU.add)
            nc.vector.scalar_tensor_tensor(
                out=lossv[:n, :], in0=suml[:n, :], scalar=-(LS / NA),
                in1=t1[:n, :], op0=ALU.mult, op1=ALU.add)
            nc.vector.tensor_tensor(out=corr[:n, :], in0=lt[:n, :],
                                    in1=mx[:n, :], op=ALU.is_equal)
            res = st_p.tile([128, 2], F32, tag="res")
            nc.vector.tensor_tensor(out=res[:n, 0:1], in0=lossv[:n, :],
                                    in1=smask[:n, tt:tt + 1], op=ALU.mult)
            nc.vector.tensor_tensor(out=res[:n, 1:2], in0=corr[:n, :],
                                    in1=smask[:n, tt:tt + 1], op=ALU.mult)
            nc.tensor.matmul(ps_out, ones128[:n, :], res[:n, :],
                             start=(tt == 0), stop=(tt == NT - 1))
        osb = st_p.tile([1, 2], F32, tag="osb")
        nc.scalar.copy(out=osb[:], in_=ps_out)
        nc.sync.dma_start(d_out[:, :], osb[:])

    nc.compile()
    return nc


# --------------------------------------------------------------------------
# host-side input preparation
# --------------------------------------------------------------------------

def prep_in_maps(inputs):
    st = np.asarray(inputs["states"])
    ac = np.asarray(inputs["actions"])
    rw = _f32(inputs["rewards"])
    qs = np.asarray(inputs["query_states"])
    ta = np.asarray(inputs["target_actions"])
    wpe = _f32(inputs["wpe"])
    emb_s = _f32(inputs["embed_state"])
    emb_a = _f32(inputs["embed_action"])
    emb_rw = _f32(inputs["embed_reward_w"])
    emb_rb = _f32(inputs["embed_reward_b"])

    sid = st[..., 0] * G + st[..., 1]          # [B, CTX]
    qid = qs[..., 0] * G + qs[..., 1]          # [B]

    # combined embedding table [87, D]
    temb = np.concatenate([emb_s, emb_a, emb_rw.reshape(1, D)], axis=0)

    # wpe_eff with reward bias folded into reward-token rows; padded to LP
    wpe_eff = np.zeros((LP, D), np.float32)
    wpe_eff[:L] = wpe[:L]
    wpe_eff[2:300:3] += emb_rb
    wpe_tok = np.concatenate([wpe_eff, wpe_eff], axis=0)   # [608, D]

    # per-layer folded weights
    layers = {}
    scale = 1.0 / np.sqrt(HD)
    vb_nz = apb_nz = mpb_nz = False
    for i in range(NL):
        g1, b1 = _f32(inputs["ln1_g"][i]), _f32(inputs["ln1_b"][i])
        w_at, b_at = _f32(inputs["attn_w"][i]), _f32(inputs["attn_b"][i])
        wq = g1[:, None] * w_at
        bq = b1 @ w_at + b_at
        wq[:, :D] *= scale
        bq = bq.copy()
        bq[:D] *= scale
        # pair q/k head-pair tiles: [q0,k0,q1,k1,...] so attention head-pair
        # hp depends only on the first 2(hp+1) output tiles of the qkv matmul
        perm = []
        for hp in range(8):
            perm.extend(range(hp * 128, (hp + 1) * 128))
            perm.extend(range(D + hp * 128, D + (hp + 1) * 128))
        wq = np.concatenate([wq[:, perm], wq[:, 2 * D:]], axis=1)
        bq = np.concatenate([bq[perm], bq[2 * D:]])
        g2, b2 = _f32(inputs["ln2_g"][i]), _f32(inputs["ln2_b"][i])
        w_fc, b_fc = _f32(inputs["fc_w"][i]), _f32(inputs["fc_b"][i])
        wf = g2[:, None] * w_fc
        bf = b2 @ w_fc + b_fc
        layers[f"w_qkv_{i}"] = _f8(_pack_pairs(wq * WS, 4, 6))
        layers[f"b_qkv_{i}"] = _f32(bq[:2 * D].reshape(16, 128).T)
        bv = bq[2 * D:]
        vb_nz |= bool(np.any(bv))
        layers[f"b_v_{i}"] = _bf16(bv.reshape(1, D))
        wap = _f32(inputs["attn_proj_w"][i])
        bap = _f32(inputs["attn_proj_b"][i])
        apb_nz |= bool(np.any(bap))
        layers[f"w_aproj_{i}"] = _f8(_pack_pairs(wap * WS, 4, 2))
        layers[f"b_aproj_{i}"] = _bf16(bap.reshape(1, D))
        layers[f"w_fc_{i}"] = _f8(_pack_pairs(wf * WS, 4, 8))
        layers[f"b_fc_{i}"] = _f32(bf.reshape(32, 128).T)
        wmp = _f32(inputs["mlp_proj_w"][i])
        bmp = _f32(inputs["mlp_proj_b"][i])
        mpb_nz |= bool(np.any(bmp))
        layers[f"w_mproj_{i}"] = _f8(_pack_pairs(wmp * WS, 16, 2))
        layers[f"b_mproj_{i}"] = _bf16(bmp.reshape(1, D))

    gf, bff = _f32(inputs["lnf_g"]), _f32(inputs["lnf_b"])
    wp = gf[:, None] * _f32(inputs["pred_w"])
    bp = bff @ _f32(inputs["pred_w"]) + _f32(inputs["pred_b"])
    predb_nz = bool(np.any(bp))
    w_pred = _bf16(wp.reshape(8, 128, NA).transpose(1, 0, 2))
    b_pred = _bf16(bp.reshape(1, NA))

    flags = (vb_nz, apb_nz, mpb_nz, predb_nz)

    # constants
    ident8 = _f8(np.eye(128, dtype=np.float32))
    ident5 = _f32(np.eye(NA))
    # transposed-scores causal mask: NEG where k (row) > q (col)
    tri = _f32(np.where(np.arange(128)[:, None] > np.arange(128)[None, :], NEG, 0.0))
    ones1 = _bf16(np.ones((1, 512), np.float32))
    ones128 = _f32(np.ones((128, 1), np.float32))

    # state-position mask [128, NT] over the padded token axis
    pos = np.arange(T) % LP
    smask_tok = ((pos < L) & (pos % 3 == 0)).astype(np.float32)
    smask = np.zeros((128, NT), np.float32)
    for tt in range(NT):
        n = NTOK[tt]
        smask[:n, tt] = smask_tok[tt * 128:tt * 128 + n]

    in_maps = []
    for c in range(NCORES):
        bs = [2 * c, 2 * c + 1]
        # one-hot embedding matrix [87, 608] (padded cols stay zero)
        m = np.zeros((EMB, T), np.float32)
        tgt = np.zeros((T, NA), np.float32)
        for s, b in enumerate(bs):
            base = s * LP
            p = np.arange(CTX)
            m[sid[b], base + 3 * p] = 1.0
            m[G * G + ac[b], base + 3 * p + 1] = 1.0
            m[EMB - 1, base + 3 * p + 2] = rw[b]
            m[qid[b], base + 300] = 1.0
            tgt[base + 3 * p, ac[b]] = 1.0
            tgt[base + 300, ta[b]] = 1.0
        im = {
            "m_embT": _bf16(m),
            "t_emb": _bf16(temb),
            "wpe": wpe_tok,
            "w_pred": w_pred,
            "tgt_oh": tgt,
            "smask": smask,
            "ident_f8": ident8,
            "ident5": ident5,
            "tri": tri,
            "ones128": ones128,
        }
        for i in range(NL):
            for k in (f"w_qkv_{i}", f"b_qkv_{i}", f"w_aproj_{i}",
                      f"w_fc_{i}", f"b_fc_{i}", f"w_mproj_{i}"):
                im[k] = layers[k]
            if flags[0]:
                im[f"b_v_{i}"] = layers[f"b_v_{i}"]
            if flags[1]:
                im[f"b_aproj_{i}"] = layers[f"b_aproj_{i}"]
            if flags[2]:
                im[f"b_mproj_{i}"] = layers[f"b_mproj_{i}"]
        if flags[3]:
            im["b_pred"] = b_pred
        if any(flags):
            im["ones1"] = ones1
        in_maps.append(im)
    return in_maps, flags


_NC_CACHE = {}


def run(inputs, trace=False):
    in_maps, flags = prep_in_maps(inputs)
    if flags not in _NC_CACHE:
        _NC_CACHE[flags] = build(flags)
    nc = _NC_CACHE[flags]
    res = run_bass_kernel_spmd(nc, in_maps, core_ids=list(range(NCORES)),
                               trace=trace)
    tot = np.zeros(2, np.float64)
    for c in range(NCORES):
        tot += res.results[c]["out"].reshape(2).astype(np.float64)
    denom = B * (CTX + 1)
    loss = np.float32(tot[0] / denom)
    acc = np.float32(tot[1] / denom)
    return (loss, acc), res


# --------------------------------------------------------------------------
# harness entry point: full inputs in, full output out
# --------------------------------------------------------------------------

def kernel(**inputs):
    """Decision-transformer forward pass on 8 TRN2 NeuronCores.

    Takes the full (unsharded) inputs of reference.setup_inputs() and
    returns (loss, acc) as float32 scalars, matching reference().
    """
    (loss, acc), _ = run(inputs, trace=False)
    return loss, acc


# revision 22
# speedup vs baseline: 1.0003x; 1.0003x over previous
"""GPT2-style decision-transformer forward pass on 8 TRN2 NeuronCores.

Data-parallel: 16 sequences -> 2 per core. Each core runs the full
4-layer transformer on its 2 sequences (602 tokens) and reduces its
loss-sum / correct-count to a [1,2] output; the host sums the 8 partials.

All large matmuls (qkv, v, fc, mproj, aproj) run in fp8e4 with
DoubleRow perf mode (two 128-deep k-tiles per matmul, 2x PE throughput).
Weights are host-scaled by WS=16 (keeps small entries out of the fp8
subnormal range) and the 1/WS is folded into the psum->sbuf copy.
Activations feeding fp8 matmuls (xnT, attnT, geluT) are stored as fp8
"pair" tiles [128, 2, T] where dim1 indexes the two k-tiles of a
DoubleRow pair.  The token axis is padded to 608 (seq stride 304) so
fp8 DoubleRow ldweights access patterns stay 4-byte aligned.

Attention uses a transposed-scores formulation in bf16: scoresT = K^T Q
per k-chunk (one matmul per chunk), exp without accumulators, then a
flipped PV (out[q, 65] = probsT^T @ [V | 1]) whose 65th column yields
the softmax row sums for free; normalization is a per-partition scaled
copy, followed by an fp8 transpose-back into attnT pair tiles.  The
pt/pv stage is software-pipelined one head-pair behind scores, woven
with the next qkv weight group's matmuls to keep the PE dense.

All problem biases are zero (setup_inputs uses zeros); the bias-via-
ones-matmul paths are only emitted when the host detects a nonzero
bias (build is cached per flag set).  qkv/fc biases ride for free in
the psum->sbuf copy ops either way.
"""

import numpy as np
import ml_dtypes
from contextlib import ExitStack

import concourse.bass as bass
import concourse.tile as tile
from concourse import bacc, mybir
from concourse.bass_utils import run_bass_kernel_spmd

F32 = mybir.dt.float32
BF16 = mybir.dt.bfloat16
FP8 = mybir.dt.float8e4
AF = mybir.ActivationFunctionType
ALU = mybir.AluOpType
DR = mybir.MatmulPerfMode.DoubleRow

B, CTX, D, H, NL, DFF, G, NA = 16, 100, 1024, 16, 4, 4096, 9, 5
L = 3 * CTX + 1          # 301
HD = D // H              # 64
LN_EPS = 1e-5
LS = 0.1
NCORES = 8
S = B // NCORES          # 2 seqs per core
LP = 304                 # per-seq padded length (4-aligned for fp8 lhsT)
T = S * LP               # 608 padded tokens per core
NTOK = [128, 128, 128, 128, 96]   # token tile sizes (padded axis)
NT = len(NTOK)
EMB = G * G + NA + 1     # 87 combined embedding rows
NEG = -60.0              # additive causal mask value (exp(-60) ~ 9e-27)
WS = 16.0                # fp8 weight pre-scale
IWS = 1.0 / WS


def _bf16(x):
    return np.asarray(x, dtype=ml_dtypes.bfloat16)


def _f8(x):
    return np.asarray(x, dtype=ml_dtypes.float8_e4m3)


def _f32(x):
    return np.ascontiguousarray(np.asarray(x, dtype=np.float32))


def _pack_pairs(w, nkp, nfg):
    """[K, F] -> [nkp, 128, nfg, 2, 512] DoubleRow pair layout.
    pack[kp, p, fg, j, f] = w[(2*kp + j)*128 + p, fg*512 + f]."""
    K, F = w.shape
    assert K == nkp * 256 and F == nfg * 512
    return np.ascontiguousarray(
        w.reshape(nkp, 2, 128, nfg, 512).transpose(0, 2, 3, 1, 4))


# --------------------------------------------------------------------------
# graph builder
# --------------------------------------------------------------------------

def build(flags=(False, False, False, False)):
    vbias_f, apbias_f, mpbias_f, predbias_f = flags
    any_ones = vbias_f or apbias_f or mpbias_f or predbias_f
    nc = bacc.Bacc("TRN2", target_bir_lowering=False, debug=False,
                   enable_asserts=True, num_devices=NCORES)

    def inp(name, shape, dt):
        return nc.dram_tensor(name, list(shape), dt, kind="ExternalInput").ap()

    d_membT = inp("m_embT", (EMB, T), BF16)
    d_temb = inp("t_emb", (EMB, D), BF16)
    d_wpe = inp("wpe", (T, D), F32)
    d_wqkv, d_bqkv, d_wap = [], [], []
    d_wfc, d_bfc, d_wmp = [], [], []
    d_bv, d_bap, d_bmp = [], [], []
    for i in range(NL):
        d_wqkv.append(inp(f"w_qkv_{i}", (4, 128, 6, 2, 512), FP8))
        d_bqkv.append(inp(f"b_qkv_{i}", (128, 16), F32))
        d_wap.append(inp(f"w_aproj_{i}", (4, 128, 2, 2, 512), FP8))
        d_wfc.append(inp(f"w_fc_{i}", (4, 128, 8, 2, 512), FP8))
        d_bfc.append(inp(f"b_fc_{i}", (128, 32), F32))
        d_wmp.append(inp(f"w_mproj_{i}", (16, 128, 2, 2, 512), FP8))
        if vbias_f:
            d_bv.append(inp(f"b_v_{i}", (1, D), BF16))
        if apbias_f:
            d_bap.append(inp(f"b_aproj_{i}", (1, D), BF16))
        if mpbias_f:
            d_bmp.append(inp(f"b_mproj_{i}", (1, D), BF16))
    d_wpred = inp("w_pred", (128, 8, NA), BF16)   # host pre-laid-out
    if predbias_f:
        d_bpred = inp("b_pred", (1, NA), BF16)
    d_tgt = inp("tgt_oh", (T, NA), F32)
    d_smask = inp("smask", (128, NT), F32)
    d_ident8 = inp("ident_f8", (128, 128), FP8)
    d_ident5 = inp("ident5", (NA, NA), F32)
    d_tri = inp("tri", (128, 128), F32)
    if any_ones:
        d_ones1 = inp("ones1", (1, 512), BF16)
    d_ones128 = inp("ones128", (128, 1), F32)
    d_out = nc.dram_tensor("out", [1, 2], F32, kind="ExternalOutput").ap()

    with tile.TileContext(nc) as tc, ExitStack() as ctx:
        # ---------------- pools
        const_p = ctx.enter_context(tc.tile_pool(name="const", bufs=1))
        pers_p = ctx.enter_context(tc.tile_pool(name="pers", bufs=1))
        w_p = ctx.enter_context(tc.tile_pool(name="w", bufs=36))
        bias_p = ctx.enter_context(tc.tile_pool(name="bias", bufs=2))
        xn_p = ctx.enter_context(tc.tile_pool(name="xn", bufs=3))
        st_p = ctx.enter_context(tc.tile_pool(name="st", bufs=24))
        pr_p = ctx.enter_context(tc.tile_pool(name="pr", bufs=28))
        ps_p = ctx.enter_context(tc.tile_pool(name="ps", bufs=8, space="PSUM"))

        def psum(pdim=128, fdim=512, dt=F32):
            t = ps_p.tile([128, 512], F32, tag="ps")
            return t[:pdim, :fdim]

        # ---------------- constants
        ident8 = const_p.tile([128, 128], FP8, tag="ident8")
        nc.sync.dma_start(ident8[:], d_ident8[:, :])
        ident5 = const_p.tile([NA, NA], F32, tag="ident5")
        nc.sync.dma_start(ident5[:], d_ident5[:, :])
        tri = const_p.tile([128, 128], F32, tag="tri")
        nc.sync.dma_start(tri[:], d_tri[:, :])
        if any_ones:
            ones1 = const_p.tile([1, 512], BF16, tag="ones1")
            nc.sync.dma_start(ones1[:], d_ones1[:, :])
        ones128 = const_p.tile([128, 1], F32, tag="ones128")
        nc.sync.dma_start(ones128[:], d_ones128[:, :])
        if predbias_f:
            bpred = const_p.tile([1, NA], BF16, tag="bpred")
            nc.sync.dma_start(bpred[:], d_bpred[:, :])
        smask = const_p.tile([128, NT], F32, tag="smask")
        nc.sync.dma_start(smask[:], d_smask[:, :])
        wpred = const_p.tile([128, 8, NA], BF16, tag="wpred")
        nc.sync.dma_start(wpred[:], d_wpred[:, :, :])
        tgt = const_p.tile([128, NT, NA], F32, tag="tgt")
        for tt in range(NT):
            n = NTOK[tt]
            nc.sync.dma_start(tgt[:n, tt, :], d_tgt[tt * 128:tt * 128 + n, :])
        eps_sb = const_p.tile([128, 1], F32, tag="eps")
        nc.vector.memset(eps_sb[:], LN_EPS)
        membT = const_p.tile([EMB, T], BF16, tag="membT")
        nc.sync.dma_start(membT[:], d_membT[:, :])
        temb = const_p.tile([EMB, D], BF16, tag="temb")
        nc.sync.dma_start(temb[:], d_temb[:, :])

        # ---------------- persistent activations
        h = [pers_p.tile([128, D], F32, tag=f"h{i}", name=f"h{i}")
             for i in range(NT)]
        # fp8 pair tiles: dim1 indexes the two k-tiles of a DoubleRow pair
        xnT = [pers_p.tile([128, 2, T], FP8, tag=f"xnT{i}", name=f"xnT{i}")
               for i in range(4)]
        qkvT = [pers_p.tile([128, T], BF16, tag=f"qkvT{i}", name=f"qkvT{i}")
                for i in range(16)]
        # v with a ones column per head (col 64): PV row-sums ride for free
        vsb = [pers_p.tile([128, 16, 65], BF16, tag=f"vsb{i}", name=f"vsb{i}")
               for i in range(6)]
        for i in range(6):
            nc.vector.memset(vsb[i][:, :, 64:65], 1.0)
        attnT = [pers_p.tile([128, 2, T], FP8, tag=f"attnT{i}", name=f"attnT{i}")
                 for i in range(4)]
        geluT = [pers_p.tile([128, 2, T], FP8, tag=f"geluT{i}", name=f"geluT{i}")
                 for i in range(16)]

        # attnT pad columns are never written by attention; zero them once
        for pp in range(4):
            nc.vector.memset(attnT[pp][:, :, L:LP], 0.0)
            nc.vector.memset(attnT[pp][:, :, LP + L:], 0.0)

        # ---------------- embedding: h = wpe_eff + M_embT.T @ T_emb
        for tt in range(NT):
            n = NTOK[tt]
            nc.sync.dma_start(h[tt][:n, :], d_wpe[tt * 128:tt * 128 + n, :])
            for half in range(2):
                ps = psum(n, 512)
                nc.tensor.matmul(ps, membT[:, tt * 128:tt * 128 + n],
                                 temb[:, half * 512:(half + 1) * 512],
                                 start=True, stop=True)
                nc.vector.tensor_tensor(
                    out=h[tt][:n, half * 512:(half + 1) * 512],
                    in0=h[tt][:n, half * 512:(half + 1) * 512],
                    in1=ps, op=ALU.add)

        # ---------------- helpers
        def ln_tile(tt, to_qkvT=False):
            """LN (pure normalize) on h[tt] -> xn fp8 -> transpose into the
            xnT fp8 pair tiles (or bf16 qkvT tiles for the final LN)."""
            n = NTOK[tt]
            st6 = st_p.tile([128, 2, 6], F32, tag="st6")
            mv = st_p.tile([128, 2], F32, tag="mv")
            std = st_p.tile([128, 1], F32, tag="std")
            inv = st_p.tile([128, 1], F32, tag="inv")
            nmi = st_p.tile([128, 1], F32, tag="nmi")
            nc.vector.bn_stats(out=st6[:n, 0, :], in_=h[tt][:n, 0:512])
            nc.vector.bn_stats(out=st6[:n, 1, :], in_=h[tt][:n, 512:1024])
            nc.vector.bn_aggr(out=mv[:n, :], in_=st6[:n, :, :])
            nc.scalar.activation(out=std[:n, :], in_=mv[:n, 1:2],
                                 func=AF.Sqrt, bias=eps_sb[:n, :], scale=1.0)
            nc.vector.reciprocal(inv[:n, :], std[:n, :])
            nc.vector.scalar_tensor_tensor(
                out=nmi[:n, :], in0=mv[:n, 0:1], scalar=-1.0,
                in1=inv[:n, :], op0=ALU.mult, op1=ALU.mult)
            xn = xn_p.tile([128, D], FP8, tag="xn")
            nc.scalar.activation(out=xn[:n, :], in_=h[tt][:n, :],
                                 func=AF.Identity, bias=nmi[:n, :],
                                 scale=inv[:n, :])
            for pp in range(4):
                # both halves of a pair transposed into one psum bank
                ps3 = ps_p.tile([128, 2, 256], F32, tag="ps", name="ps_tr")
                for j in range(2):
                    dc = 2 * pp + j
                    nc.tensor.matmul(ps3[:, j, :n],
                                     xn[:n, dc * 128:(dc + 1) * 128],
                                     ident8[:n, :n],
                                     start=(j == 0), stop=(j == 1),
                                     skip_group_check=True)
                c0 = tt * 128
                if to_qkvT:
                    for j in range(2):
                        nc.vector.tensor_copy(
                            out=qkvT[2 * pp + j][:, c0:c0 + n],
                            in_=ps3[:, j, :n])
                elif pp % 2 == 0:
                    nc.vector.tensor_copy(out=xnT[pp][:, :, c0:c0 + n],
                                          in_=ps3[:, :, :n])
                else:
                    nc.scalar.copy(out=xnT[pp][:, :, c0:c0 + n],
                                   in_=ps3[:, :, :n])

        def featT_group_weights(d_w, fg):
            wt = []
            for kp in range(4):
                w = w_p.tile([128, 2, 512], FP8, tag="w")
                nc.sync.dma_start(w[:], d_w[kp, :, fg, :, :])
                wt.append(w)
            return wt

        def featT_group_mms(wt, outT, fg, bias_sb, act_func, fs_range):
            """DoubleRow featT matmuls: out f-tile fq = 4*fg+fs."""
            for fs in fs_range:
                fq = fg * 4 + fs
                ps0 = psum(128, 512)
                ps1 = psum(128, 96)
                for kp in range(4):
                    lhs = wt[kp][:, :, fs * 128:(fs + 1) * 128]
                    nc.tensor.matmul(ps0, lhs, xnT[kp][:, :, 0:512],
                                     start=(kp == 0), stop=(kp == 3),
                                     perf_mode=DR)
                    nc.tensor.matmul(ps1, lhs, xnT[kp][:, :, 512:608],
                                     start=(kp == 0), stop=(kp == 3),
                                     perf_mode=DR)
                for ps, sl in ((ps0, slice(0, 512)), (ps1, slice(512, 608))):
                    if act_func is None:
                        nc.vector.tensor_scalar(
                            out=outT[fq][:, sl], in0=ps,
                            scalar1=IWS, scalar2=bias_sb[:, fq:fq + 1],
                            op0=ALU.mult, op1=ALU.add)
                    else:
                        # fc path: out tile fq of geluT pairs
                        nc.scalar.activation(
                            out=outT[fq // 2][:, fq % 2, sl], in_=ps,
                            func=act_func, bias=bias_sb[:, fq:fq + 1],
                            scale=IWS)

        def proj_residual(d_w, inT, nkp, tail=None, bias_sb=None):
            """h += (inT.T @ W) / WS [+ b];  inT fp8 pair tiles."""
            for nh in range(2):
                pss = [psum(NTOK[tt], 512) for tt in range(NT)]
                for blk in range(0, nkp, 8):
                    be = min(blk + 8, nkp)
                    wt = []
                    for kp in range(blk, be):
                        w = w_p.tile([128, 2, 512], FP8, tag="w")
                        nc.sync.dma_start(w[:], d_w[kp, :, nh, :, :])
                        wt.append(w)
                    for tt in range(NT):
                        n = NTOK[tt]
                        for j, kp in enumerate(range(blk, be)):
                            nc.tensor.matmul(
                                pss[tt],
                                inT[kp][:, :, tt * 128:tt * 128 + n],
                                wt[j], start=(kp == 0), stop=(kp == nkp - 1),
                                perf_mode=DR)
                for tt in range(NT):
                    n = NTOK[tt]
                    if bias_sb is not None:
                        # rarely-taken generic path: bias via ones matmul
                        nc.tensor.matmul(pss[tt], ones1[0:1, :n],
                                         bias_sb[0:1, nh * 512:(nh + 1) * 512],
                                         start=False, stop=True,
                                         skip_group_check=True)
                    nc.vector.scalar_tensor_tensor(
                        out=h[tt][:n, nh * 512:(nh + 1) * 512],
                        in0=pss[tt], scalar=IWS,
                        in1=h[tt][:n, nh * 512:(nh + 1) * 512],
                        op0=ALU.mult, op1=ALU.add)
                    if nh == 1 and tail is not None:
                        tail(tt)

        # ---------------- transformer layers
        SEQCH = [(0, 128), (128, 128), (256, 45)]   # per-seq k-chunks

        for li in range(NL):
            bqkv = bias_p.tile([128, 16], F32, tag="bqkv")
            nc.sync.dma_start(bqkv[:], d_bqkv[li][:, :])
            bfc = bias_p.tile([128, 32], F32, tag="bfc")
            nc.sync.dma_start(bfc[:], d_bfc[li][:, :])
            bv = bap = bmp = None
            if vbias_f:
                bv = bias_p.tile([1, D], BF16, tag="bv")
                nc.sync.dma_start(bv[:], d_bv[li][:, :])
            if apbias_f:
                bap = bias_p.tile([1, D], BF16, tag="bap")
                nc.sync.dma_start(bap[:], d_bap[li][:, :])
            if mpbias_f:
                bmp = bias_p.tile([1, D], BF16, tag="bmp")
                nc.sync.dma_start(bmp[:], d_bmp[li][:, :])

            # ---- ln1; v first (swapped DoubleRow matmul producing
            # V[tok, feat] per-seq-chunk), then q,k via featT matmuls
            if li == 0:
                for tt in range(NT):
                    ln_tile(tt)
            for nh in range(2):
                wv = []
                for kp in range(4):
                    w = w_p.tile([128, 2, 512], FP8, tag="w", name="wv")
                    nc.sync.dma_start(w[:], d_wqkv[li][kp, :, 4 + nh, :, :])
                    wv.append(w)
                for s in range(2):
                    for kc in range(3):
                        c0, cw = SEQCH[kc]
                        col = s * LP + c0
                        ps = psum(cw, 512)
                        for kp in range(4):
                            nc.tensor.matmul(ps, xnT[kp][:, :, col:col + cw],
                                             wv[kp], start=(kp == 0),
                                             stop=(kp == 3), perf_mode=DR)
                        if vbias_f:
                            nc.tensor.matmul(ps, ones1[0:1, :cw],
                                             bv[0:1, nh * 512:(nh + 1) * 512],
                                             start=False, stop=True,
                                             skip_group_check=True)
                        nc.vector.tensor_scalar_mul(
                            out=vsb[s * 3 + kc][:cw, nh * 8:(nh + 1) * 8, 0:64],
                            in0=ps.rearrange("p (h f) -> p h f", h=8),
                            scalar1=IWS)

            # ---- attention (transposed-scores form), woven with the qkv
            # matmul groups: head-pair hp needs only qkvT tiles 2hp, 2hp+1.
            # scoresT[k, q] = K^T Q per k-chunk; exp -> probsT (bf16, no
            # accumulator); PV-flipped out[q, 65] = probsT^T @ [V | 1] gives
            # the row sums for free in column 64; per-partition normalize on
            # the psum->sbuf copy; fp8 transpose-back into attnT pairs.
            QWS = [128, 128, 45]

            def attn_scores(hp, s):
                """scoresT + exp for both heads of pair hp, seq s."""
                off = s * LP
                probsT = {}
                for kc in range(3):
                    kcw = QWS[kc]
                    qn = L - kc * 128      # q range [kc*128, L)
                    for sub in range(2):
                        bp = sub * 64
                        ps_s = psum(kcw, qn)
                        nc.tensor.matmul(
                            ps_s,
                            qkvT[2 * hp + 1][bp:bp + 64,
                                             off + kc * 128: off + kc * 128 + kcw],
                            qkvT[2 * hp][bp:bp + 64, off + kc * 128: off + L],
                            start=True, stop=True)
                        nc.vector.tensor_tensor(
                            out=ps_s[:, 0:kcw], in0=ps_s[:, 0:kcw],
                            in1=tri[:kcw, :kcw], op=ALU.add)
                        pb = pr_p.tile([128, L], BF16, tag="probs")
                        nc.scalar.activation(out=pb[:kcw, :qn], in_=ps_s,
                                             func=AF.Exp)
                        probsT[sub, kc] = pb
                return (probsT,)

            def attn_pt_pv(hp, s, probsT):
                """flipped PV + rowsum, normalize, transpose-back; hp, seq s."""
                off = s * LP
                # PV: one bank per sub holding 3 q-chunks of [qw, 65]
                avs, rsi, aq = [], [], {}
                for sub in range(2):
                    av = ps_p.tile([128, 3, 65], F32, tag="ps", name="ps_av")
                    for qc in range(3):
                        qw = QWS[qc]
                        for kc in range(qc + 1):
                            kcw = QWS[kc]
                            lo = (qc - kc) * 128
                            nc.tensor.matmul(
                                av[:qw, qc, :],
                                probsT[sub, kc][:kcw, lo:lo + qw],
                                vsb[s * 3 + kc][:kcw, 2 * hp + sub, :],
                                start=(qc == 0 and kc == 0), stop=(kc == qc),
                                skip_group_check=True)
                    rs = st_p.tile([128, 3], F32, tag="rs")
                    nc.vector.tensor_copy(out=rs[:, :], in_=av[:, :, 64])
                    ri = st_p.tile([128, 3], F32, tag="ri")
                    nc.vector.reciprocal(ri[:, :], rs[:, :])
                    avs.append(av)
                    rsi.append(ri)
                    for qc in range(3):
                        qw = QWS[qc]
                        a = pr_p.tile([128, 64], FP8, tag="aq")
                        nc.scalar.activation(out=a[:qw, :],
                                             in_=av[:qw, qc, 0:64],
                                             func=AF.Copy,
                                             scale=rsi[sub][:qw, qc:qc + 1])
                        aq[sub, qc] = a
                # transpose-back: one bank, sub0 at partitions 0:64, sub1 at
                # 64:128; q-chunks at disjoint columns (start only on first)
                tb = ps_p.tile([128, 384], F32, tag="ps", name="ps_tb")
                for sub in range(2):
                    bp = sub * 64
                    for qc in range(3):
                        qw = QWS[qc]
                        nc.tensor.matmul(
                            tb[bp:bp + 64, qc * 128: qc * 128 + qw],
                            aq[sub, qc][:qw, :], ident8[:qw, :qw],
                            start=(qc == 0), stop=(qc == 2),
                            skip_group_check=True)
                # tb columns 0:301 are exactly q 0:301 (128-wide slots align)
                nc.vector.tensor_copy(
                    out=attnT[hp // 2][:, hp % 2, off:off + L],
                    in_=tb[:, 0:L])

            wt = featT_group_weights(d_wqkv[li], 0)
            featT_group_mms(wt, qkvT, 0, bqkv, None, range(4))
            pend = None   # software-pipeline pt/pv one head-pair behind
            for fg in range(4):
                for j, hp in enumerate((2 * fg, 2 * fg + 1)):
                    st0 = attn_scores(hp, 0)
                    st1 = attn_scores(hp, 1)
                    # interleave half of next qkv group's matmuls
                    if fg < 3:
                        if j == 0:
                            wt = featT_group_weights(d_wqkv[li], fg + 1)
                            featT_group_mms(wt, qkvT, fg + 1, bqkv, None, (0, 1))
                        else:
                            featT_group_mms(wt, qkvT, fg + 1, bqkv, None, (2, 3))
                    if pend is not None:
                        attn_pt_pv(*pend[0])
                        attn_pt_pv(*pend[1])
                    pend = ((hp, 0) + st0, (hp, 1) + st1)
            attn_pt_pv(*pend[0])
            attn_pt_pv(*pend[1])

            # ---- attn proj + residual (ln2 fused into the tail)
            proj_residual(d_wap[li], attnT, 4, tail=ln_tile,
                          bias_sb=bap if apbias_f else None)

            # ---- mlp (next-layer ln1 / lnf fused into mproj's tail)
            for fg in range(8):
                wt = featT_group_weights(d_wfc[li], fg)
                featT_group_mms(wt, geluT, fg, bfc, AF.Gelu_apprx_tanh,
                                range(4))
            last = (li == NL - 1)
            proj_residual(d_wmp[li], geluT, 16,
                          tail=(lambda tt: ln_tile(tt, to_qkvT=True))
                          if last else ln_tile,
                          bias_sb=bmp if mpbias_f else None)

        # ---------------- logits + loss (lnf output sits in qkvT bf16)
        lg = const_p.tile([NA, T], F32, tag="logits")
        for tch, (t0, tn) in enumerate(((0, 512), (512, 96))):
            ps = psum(NA, tn)
            for dc in range(8):
                nc.tensor.matmul(ps, wpred[:, dc, :], qkvT[dc][:, t0:t0 + tn],
                                 start=(dc == 0), stop=(dc == 7))
            if predbias_f:
                nc.tensor.matmul(ps, bpred[0:1, :], ones1[0:1, :tn],
                                 start=False, stop=True,
                                 skip_group_check=True)
            nc.scalar.copy(out=lg[:, t0:t0 + tn], in_=ps)

        ps_out = psum(1, 2)
        for tt in range(NT):
            n = NTOK[tt]
            ps_t = psum(n, NA)
            nc.tensor.matmul(ps_t, lg[:, tt * 128:tt * 128 + n],
                             ident5[:, :], start=True, stop=True)
            sexp = st_p.tile([128, 1], F32, tag="sexp")
            lse = st_p.tile([128, 1], F32, tag="lse")
            lt = st_p.tile([128, 1], F32, tag="lt")
            suml = st_p.tile([128, 1], F32, tag="suml")
            mx = st_p.tile([128, 1], F32, tag="mx")
            t1 = st_p.tile([128, 1], F32, tag="t1")
            lossv = st_p.tile([128, 1], F32, tag="lossv")
            corr = st_p.tile([128, 1], F32, tag="corr")
            scr5 = st_p.tile([128, NA], F32, tag="scr5")
            scr5b = st_p.tile([128, NA], F32, tag="scr5b")
            nc.scalar.activation(out=scr5[:n, :], in_=ps_t, func=AF.Exp,
                                 accum_out=sexp[:n, :])
            nc.scalar.activation(out=lse[:n, :], in_=sexp[:n, :], func=AF.Ln)
            nc.vector.scalar_tensor_tensor(
                out=scr5b[:n, :], in0=ps_t, scalar=1.0,
                in1=tgt[:n, tt, :], op0=ALU.mult, op1=ALU.mult,
                accum_out=lt[:n, :])
            nc.vector.tensor_reduce(out=suml[:n, :], in_=ps_t,
                                    axis=mybir.AxisListType.X, op=ALU.add)
            nc.vector.tensor_reduce(out=mx[:n, :], in_=ps_t,
                                    axis=mybir.AxisListType.X, op=ALU.max)
            nc.vector.scalar_tensor_tensor(
                out=t1[:n, :], in0=lt[:n, :], scalar=-(1.0 - LS),
                in1=lse[:n, :], op0=ALU.mult, op1=ALU.add)
            nc.vector.scalar_tensor_tensor(
                out=lossv[:n, :], in0=suml[:n, :], scalar=-(LS / NA),
                in1=t1[:n, :], op0=ALU.mult, op1=ALU.add)
            nc.vector.tensor_tensor(out=corr[:n, :], in0=lt[:n, :],
                                    in1=mx[:n, :], op=ALU.is_equal)
            res = st_p.tile([128, 2], F32, tag="res")
            nc.vector.tensor_tensor(out=res[:n, 0:1], in0=lossv[:n, :],
                                    in1=smask[:n, tt:tt + 1], op=ALU.mult)
            nc.vector.tensor_tensor(out=res[:n, 1:2], in0=corr[:n, :],
                                    in1=smask[:n, tt:tt + 1], op=ALU.mult)
            nc.tensor.matmul(ps_out, ones128[:n, :], res[:n, :],
                             start=(tt == 0), stop=(tt == NT - 1))
        osb = st_p.tile([1, 2], F32, tag="osb")
        nc.scalar.copy(out=osb[:], in_=ps_out)
        nc.sync.dma_start(d_out[:, :], osb[:])

    nc.compile()
    return nc


# --------------------------------------------------------------------------
# host-side input preparation
# --------------------------------------------------------------------------

def prep_in_maps(inputs):
    st = np.asarray(inputs["states"])
    ac = np.asarray(inputs["actions"])
    rw = _f32(inputs["rewards"])
    qs = np.asarray(inputs["query_states"])
    ta = np.asarray(inputs["target_actions"])
    wpe = _f32(inputs["wpe"])
    emb_s = _f32(inputs["embed_state"])
    emb_a = _f32(inputs["embed_action"])
    emb_rw = _f32(inputs["embed_reward_w"])
    emb_rb = _f32(inputs["embed_reward_b"])

    sid = st[..., 0] * G + st[..., 1]          # [B, CTX]
    qid = qs[..., 0] * G + qs[..., 1]          # [B]

    # combined embedding table [87, D]
    temb = np.concatenate([emb_s, emb_a, emb_rw.reshape(1, D)], axis=0)

    # wpe_eff with reward bias folded into reward-token rows; padded to LP
    wpe_eff = np.zeros((LP, D), np.float32)
    wpe_eff[:L] = wpe[:L]
    wpe_eff[2:300:3] += emb_rb
    wpe_tok = np.concatenate([wpe_eff, wpe_eff], axis=0)   # [608, D]

    # per-layer folded weights
    layers = {}
    scale = 1.0 / np.sqrt(HD)
    vb_nz = apb_nz = mpb_nz = False
    for i in range(NL):
        g1, b1 = _f32(inputs["ln1_g"][i]), _f32(inputs["ln1_b"][i])
        w_at, b_at = _f32(inputs["attn_w"][i]), _f32(inputs["attn_b"][i])
        wq = g1[:, None] * w_at
        bq = b1 @ w_at + b_at
        wq[:, :D] *= scale
        bq = bq.copy()
        bq[:D] *= scale
        # pair q/k head-pair tiles: [q0,k0,q1,k1,...] so attention head-pair
        # hp depends only on the first 2(hp+1) output tiles of the qkv matmul
        perm = []
        for hp in range(8):
            perm.extend(range(hp * 128, (hp + 1) * 128))
            perm.extend(range(D + hp * 128, D + (hp + 1) * 128))
        wq = np.concatenate([wq[:, perm], wq[:, 2 * D:]], axis=1)
        bq = np.concatenate([bq[perm], bq[2 * D:]])
        g2, b2 = _f32(inputs["ln2_g"][i]), _f32(inputs["ln2_b"][i])
        w_fc, b_fc = _f32(inputs["fc_w"][i]), _f32(inputs["fc_b"][i])
        wf = g2[:, None] * w_fc
        bf = b2 @ w_fc + b_fc
        layers[f"w_qkv_{i}"] = _f8(_pack_pairs(wq * WS, 4, 6))
        layers[f"b_qkv_{i}"] = _f32(bq[:2 * D].reshape(16, 128).T)
        bv = bq[2 * D:]
        vb_nz |= bool(np.any(bv))
        layers[f"b_v_{i}"] = _bf16(bv.reshape(1, D))
        wap = _f32(inputs["attn_proj_w"][i])
        bap = _f32(inputs["attn_proj_b"][i])
        apb_nz |= bool(np.any(bap))
        layers[f"w_aproj_{i}"] = _f8(_pack_pairs(wap * WS, 4, 2))
        layers[f"b_aproj_{i}"] = _bf16(bap.reshape(1, D))
        layers[f"w_fc_{i}"] = _f8(_pack_pairs(wf * WS, 4, 8))
        layers[f"b_fc_{i}"] = _f32(bf.reshape(32, 128).T)
        wmp = _f32(inputs["mlp_proj_w"][i])
        bmp = _f32(inputs["mlp_proj_b"][i])
        mpb_nz |= bool(np.any(bmp))
        layers[f"w_mproj_{i}"] = _f8(_pack_pairs(wmp * WS, 16, 2))
        layers[f"b_mproj_{i}"] = _bf16(bmp.reshape(1, D))

    gf, bff = _f32(inputs["lnf_g"]), _f32(inputs["lnf_b"])
    wp = gf[:, None] * _f32(inputs["pred_w"])
    bp = bff @ _f32(inputs["pred_w"]) + _f32(inputs["pred_b"])
    predb_nz = bool(np.any(bp))
    w_pred = _bf16(wp.reshape(8, 128, NA).transpose(1, 0, 2))
    b_pred = _bf16(bp.reshape(1, NA))

    flags = (vb_nz, apb_nz, mpb_nz, predb_nz)

    # constants
    ident8 = _f8(np.eye(128, dtype=np.float32))
    ident5 = _f32(np.eye(NA))
    # transposed-scores causal mask: NEG where k (row) > q (col)
    tri = _f32(np.where(np.arange(128)[:, None] > np.arange(128)[None, :], NEG, 0.0))
    ones1 = _bf16(np.ones((1, 512), np.float32))
    ones128 = _f32(np.ones((128, 1), np.float32))

    # state-position mask [128, NT] over the padded token axis
    pos = np.arange(T) % LP
    smask_tok = ((pos < L) & (pos % 3 == 0)).astype(np.float32)
    smask = np.zeros((128, NT), np.float32)
    for tt in range(NT):
        n = NTOK[tt]
        smask[:n, tt] = smask_tok[tt * 128:tt * 128 + n]

    in_maps = []
    for c in range(NCORES):
        bs = [2 * c, 2 * c + 1]
        # one-hot embedding matrix [87, 608] (padded cols stay zero)
        m = np.zeros((EMB, T), np.float32)
        tgt = np.zeros((T, NA), np.float32)
        for s, b in enumerate(bs):
            base = s * LP
            p = np.arange(CTX)
            m[sid[b], base + 3 * p] = 1.0
            m[G * G + ac[b], base + 3 * p + 1] = 1.0
            m[EMB - 1, base + 3 * p + 2] = rw[b]
            m[qid[b], base + 300] = 1.0
            tgt[base + 3 * p, ac[b]] = 1.0
            tgt[base + 300, ta[b]] = 1.0
        im = {
            "m_embT": _bf16(m),
            "t_emb": _bf16(temb),
            "wpe": wpe_tok,
            "w_pred": w_pred,
            "tgt_oh": tgt,
            "smask": smask,
            "ident_f8": ident8,
            "ident5": ident5,
            "tri": tri,
            "ones128": ones128,
        }
        for i in range(NL):
            for k in (f"w_qkv_{i}", f"b_qkv_{i}", f"w_aproj_{i}",
                      f"w_fc_{i}", f"b_fc_{i}", f"w_mproj_{i}"):
                im[k] = layers[k]
            if flags[0]:
                im[f"b_v_{i}"] = layers[f"b_v_{i}"]
            if flags[1]:
                im[f"b_aproj_{i}"] = layers[f"b_aproj_{i}"]
            if flags[2]:
                im[f"b_mproj_{i}"] = layers[f"b_mproj_{i}"]
        if flags[3]:
            im["b_pred"] = b_pred
        if any(flags):
            im["ones1"] = ones1
        in_maps.append(im)
    return in_maps, flags


_NC_CACHE = {}


def run(inputs, trace=False):
    in_maps, flags = prep_in_maps(inputs)
    if flags not in _NC_CACHE:
        _NC_CACHE[flags] = build(flags)
    nc = _NC_CACHE[flags]
    res = run_bass_kernel_spmd(nc, in_maps, core_ids=list(range(NCORES)),
                               trace=trace)
    tot = np.zeros(2, np.float64)
    for c in range(NCORES):
        tot += res.results[c]["out"].reshape(2).astype(np.float64)
    denom = B * (CTX + 1)
    loss = np.float32(tot[0] / denom)
    acc = np.float32(tot[1] / denom)
    return (loss, acc), res


# --------------------------------------------------------------------------
# harness entry point: full inputs in, full output out
# --------------------------------------------------------------------------

def kernel(**inputs):
    """Decision-transformer forward pass on 8 TRN2 NeuronCores.

    Takes the full (unsharded) inputs of reference.setup_inputs() and
    returns (loss, acc) as float32 scalars, matching reference().
    """
    (loss, acc), _ = run(inputs, trace=False)
    return loss, acc
